# revision 1
# baseline (speedup 1.0000x reference)
"""Causal self-attention kernel for 8 Trainium2 NeuronCores.

Sharding: core c -> (batch b = c//2, head-group g = c%2). Each core computes
the attention output contribution of 8 heads for one batch element:
    P_c = (sum_{h in group} softmax(Q_h K_h^T / 8 + causal) V_h) @ WO
Host epilogue: out[b] = P_{2b} + P_{2b+1} + (sum_h bV_h) @ WO + 16*bO
(the V-bias commutes through softmax normalization: softmax rows sum to 1).

v3 design notes:
  - Projections and scores are fp16 (fp8 anywhere in the Q/K/score path or
    plain-fp8 V pushes the max-rel error past the 2e-2 gate; measured).
  - ET (exp of scores) is fp8e4m3: halves the ScalarE exp cost (write-
    bandwidth-limited: 1008ns vs 1370ns per 1024-wide tile) and enables
    DoubleRow A@V. V is stored as fp8 V8 plus an fp8 residual R8 with
    V ~= V8+R8 (0.25% effective error); the A@V accumulates V8-pairs and
    R8-pairs per kt-pair via DoubleRow at the same PE cycles as fp16 ZT.
  - QK projections stream 1024 moving columns (a chunk pair) per matmul
    to halve projection instruction count.
  - 1/l realized as a tensor_tensor divide against a DRAM-bounce partition
    broadcast of l (no reciprocal op; DVE RECIPROCAL measured 6.5us/call,
    reciprocal_approx_fast NaNs on rare inputs).
  - Per-head-pair tail hooks: chunk qc's normalization + out-projection
    interleaves into attention(qc+1); the final chunk tails interleave into
    its own attention loop, removing the serial end-of-kernel tail.

Per core, streamed per 512-row chunk:
  xT[d, s]        x f16, PE-transposed on load (16 transposes gathered per
                  2-bank PSUM tile -> one DVE copy per half)
  QT[dh, s], KT   = W.T @ xT over 1024-wide chunk pairs; bias added by the
                  DVE during PSUM eviction
  V8/R8[s, 66/hd] = x @ WV split into fp8 main + fp8 residual, plus a ones
                  column (row 64 of A@V accumulates the denominator l) and
                  a zero pad column (DoubleRow needs even weight width)
  ST[k, q]        two heads packed per PE pass via tile_position (0,0)/(64,0)
                  into one [128,1024] PSUM tile
  ET fp8          = exp(ST/8) on ScalarE into a [128,2048] kt-pair tile;
                  fully-masked column blocks skip exp and are zero-filled;
                  the diagonal 128-block is masked by a gpsimd affine_select
  ZT[66,q]        += V8_pair.T @ ET_pair + R8_pair.T @ ET_pair (DoubleRow,
                  one PSUM accumulation group), pipelined 2 pairs behind ST
  out             = (sum_h ZT_h / l_h).T @ WO, q-tile pairs packed via
                  tile_position with WO/Z duplicated in both partition halves
"""
import numpy as np

B, S, D, H, DH = 4, 2048, 1024, 16, 64
HPC = 8            # heads per core
GD = HPC * DH      # 512 = group width
NCORES = 8
NQ = S // 512      # 4 q/s chunks of 512
NKT = S // 128     # 16 k-tiles
NDT = D // 128     # 8 d-tiles

_prog = {}


def bass_ap_3d(tile_t, offset, stride, n, inner):
    """AP view [128p, n, inner] over a tile's free dim: col = offset + i*stride + c."""
    import concourse.bass as bass
    ap = tile_t[:]
    return bass.AP(ap.tensor, ap.offset + offset,
                   [ap.ap[0], [stride, n], [1, inner]])


def _bcast_ap(tile_t, row, col, nparts, width):
    """Partition-step-0 AP reading (row, col:col+width) replicated nparts times."""
    import concourse.bass as bass
    ap = tile_t[:]
    pstep = ap.ap[0][0]
    return bass.AP(ap.tensor, ap.offset + row * pstep + col,
                   [[0, nparts], [1, width]])


def _build():
    import concourse.bacc as bacc
    import concourse.tile as tile
    from concourse import mybir

    f32 = mybir.dt.float32
    f16 = mybir.dt.float16
    f8 = mybir.dt.float8e4
    AF = mybir.ActivationFunctionType
    ALU = mybir.AluOpType
    DR = mybir.MatmulPerfMode.DoubleRow

    nc = bacc.Bacc(None, target_bir_lowering=False, debug=False)
    x = nc.dram_tensor("x", [S, D], f16, kind="ExternalInput")
    wq = nc.dram_tensor("wq", [D, GD], f16, kind="ExternalInput")
    wk = nc.dram_tensor("wk", [D, GD], f16, kind="ExternalInput")
    wv = nc.dram_tensor("wv", [D, GD], f16, kind="ExternalInput")
    bq = nc.dram_tensor("bq", [1, GD], f16, kind="ExternalInput")
    bk = nc.dram_tensor("bk", [1, GD], f16, kind="ExternalInput")
    wo = nc.dram_tensor("wo", [DH, D], f16, kind="ExternalInput")
    out = nc.dram_tensor("out", [S, D], f32, kind="ExternalOutput")

    with tile.TileContext(nc) as tc:
        with tc.tile_pool(name="const", bufs=1) as constp, \
             tc.tile_pool(name="big", bufs=1) as bigp:
            idt = constp.tile([128, 128], f16, tag="idt")
            from concourse.masks import make_identity
            make_identity(nc, idt[:])
            import concourse.bass as bass
            bq_t = constp.tile([128, 4], f32, tag="bq_t")
            bk_t = constp.tile([128, 4], f32, tag="bk_t")
            nc.gpsimd.dma_start(bq_t[:], bass.AP(bq, 0, [[1, 128], [128, 4]]))
            nc.gpsimd.dma_start(bk_t[:], bass.AP(bk, 0, [[1, 128], [128, 4]]))
            wo_sb = constp.tile([128, D], f16, tag="wo_sb")
            nc.gpsimd.dma_start(wo_sb[0:DH, :], wo[:])
            nc.gpsimd.dma_start(wo_sb[DH:2 * DH, :], wo[:])

            # persistent per-core tensors
            xt_all = bigp.tile([128, NDT * S], f16, tag="xt")  # d-tile j at cols j*S
            qt_all = bigp.tile([128, 4 * S], f16, tag="qt")    # m-tile m at cols m*S
            kt_all = bigp.tile([128, 4 * S], f16, tag="kt")
            vt_all = bigp.tile([128, NKT * 528], f8, tag="vt")  # V8 + ones col
            rt_all = bigp.tile([128, NKT * 528], f8, tag="rt")  # fp8 residual
            zsum = bigp.tile([DH, S], f32, tag="zsum")

            with tc.tile_pool(name="wts", bufs=1) as wtp, \
                 tc.tile_pool(name="xs", bufs=4) as xsp, \
                 tc.tile_pool(name="et", bufs=5) as etp, \
                 tc.tile_pool(name="ztall", bufs=2) as zta_p, \
                 tc.tile_pool(name="rld", bufs=4, space="DRAM") as rldp, \
                 tc.tile_pool(name="lbs", bufs=3) as lbsp, \
                 tc.tile_pool(name="zn", bufs=2) as znp, \
                 tc.tile_pool(name="zr", bufs=2) as zrp, \
                 tc.tile_pool(name="osb", bufs=2) as osbp, \
                 tc.tile_pool(name="stp", bufs=2, space="PSUM") as stp, \
                 tc.tile_pool(name="ppp", bufs=2, space="PSUM") as ppp, \
                 tc.tile_pool(name="ztp", bufs=2, space="PSUM") as ztp:
                # one DMA per weight tensor: d-tile k lands at cols k*512
                # (24 serialized per-tile DMAs on the gpsimd queue delayed
                # the first projections by ~15us)
                wq_all = wtp.tile([128, NDT * GD], f16, tag="wq_all")
                wk_all = wtp.tile([128, NDT * GD], f16, tag="wk_all")
                wv_all = wtp.tile([128, NDT * GD], f16, tag="wv_all")
                for (w_all, w_dram) in ((wq_all, wq), (wk_all, wk),
                                        (wv_all, wv)):
                    nc.gpsimd.dma_start(
                        bass_ap_3d(w_all, 0, GD, NDT, GD),
                        bass.AP(w_dram, 0, [[GD, 128], [128 * GD, NDT],
                                            [1, GD]]))

                def transpose_chunk(nq):
                    # PE transpose (DMA XBAR transpose measured 4x slower:
                    # transfers serialize on one queue)
                    xss = []
                    for st4 in range(4):
                        srow = nq * 512 + st4 * 128
                        xs = xsp.tile([128, D], f16, tag="xs", name="xs")
                        nc.sync.dma_start(xs[:], x[srow:srow + 128, :])
                        xss.append(xs)
                    for jj in range(4):
                        pt = ppp.tile([128, 1024], f16, tag="pp", name="pt")
                        for j2 in range(2):
                            j = jj * 2 + j2
                            for st4 in range(4):
                                nc.tensor.transpose(
                                    pt[:, j2 * 512 + st4 * 128:
                                       j2 * 512 + (st4 + 1) * 128],
                                    xss[st4][:, j * 128:(j + 1) * 128], idt[:])
                        dst = bass_ap_3d(xt_all, (jj * 2) * S + nq * 512, S, 2, 512)
                        srcap = bass_ap_3d(pt, 0, 512, 2, 512)
                        nc.vector.tensor_copy(dst, srcap)

                def vproj_chunk(nq):
                    for m in range(4):
                        st = nq * 4 + m
                        ps = ppp.tile([128, 512], f32, tag="pp", name="ps")
                        for k in range(NDT):
                            nc.tensor.matmul(
                                ps[:],
                                xt_all[:, k * S + st * 128: k * S + (st + 1) * 128],
                                wv_all[:, k * GD:(k + 1) * GD],
                                start=(k == 0), stop=(k == NDT - 1))
                        dst = bass_ap_3d(vt_all, st * 528, 66, HPC, DH)
                        srcap = bass_ap_3d(ps, 0, DH, HPC, DH)
                        nc.vector.tensor_copy(dst, srcap)
                        # residual: R8 = psum - float(V8)
                        rdst = bass_ap_3d(rt_all, st * 528, 66, HPC, DH)
                        nc.vector.tensor_tensor(rdst, srcap, dst, op=ALU.subtract)
                        # ones column for l (V8 only) and zero pads
                        nc.gpsimd.memset(
                            bass_ap_3d(vt_all, st * 528 + DH, 66, HPC, 2), 0.0)
                        nc.gpsimd.memset(
                            bass_ap_3d(vt_all, st * 528 + DH, 66, HPC, 1), 1.0)
                        nc.gpsimd.memset(
                            bass_ap_3d(rt_all, st * 528 + DH, 66, HPC, 2), 0.0)

                def proj_pair(p):
                    # chunks 2p, 2p+1: transposes, then Q/K/V projections
                    # (matmul moving size is ISA-capped at 512 elements)
                    transpose_chunk(2 * p)
                    transpose_chunk(2 * p + 1)
                    for nq in (2 * p, 2 * p + 1):
                        for (w_all, b_t, dest) in ((wq_all, bq_t, qt_all),
                                                   (wk_all, bk_t, kt_all)):
                            for m in range(4):
                                ps = ppp.tile([128, 512], f32, tag="pp",
                                              name="ps")
                                for k in range(NDT):
                                    nc.tensor.matmul(
                                        ps[:],
                                        w_all[:, k * GD + m * 128:
                                              k * GD + (m + 1) * 128],
                                        xt_all[:, k * S + nq * 512:
                                               k * S + (nq + 1) * 512],
                                        start=(k == 0), stop=(k == NDT - 1))
                                nc.vector.tensor_scalar_add(
                                    dest[:, m * S + nq * 512:
                                         m * S + (nq + 1) * 512],
                                    ps[:], b_t[:, m:m + 1])
                        vproj_chunk(nq)

                ztalls = {}

                def attention(qc, hooks=None):
                    hooks = hooks or {}
                    ktiles = 4 * qc + 4
                    npairs = ktiles // 2
                    ztall = zta_p.tile([65, HPC * 512], f32, tag="ztall",
                                       name=f"ztall{qc}")
                    ztalls[qc] = ztall
                    for hp in range(4):
                        zt0 = ztp.tile([66, 512], f32, tag="zt", name="zt0")
                        zt1 = ztp.tile([66, 512], f32, tag="zt", name="zt1")
                        pending = []  # completed ET pair tiles, lag 2 pairs

                        def flush_zt(lag):
                            while len(pending) > lag:
                                pp, pet = pending.pop(0)
                                # columns left of the first kt's diagonal are
                                # zero in both ET subtiles: skip them (valid
                                # because start=True already zeroed the full
                                # region on the first pair, where jz=0)
                                jz = max(2 * pp - 4 * qc, 0)
                                q0 = jz * 128
                                for half, zt in ((0, zt0), (1, zt1)):
                                    eap = bass_ap_3d(pet, half * 512 + q0,
                                                     1024, 2, 512 - q0)
                                    voff = ((2 * pp) * 528
                                            + (2 * hp + half) * 66)
                                    nc.tensor.matmul(
                                        zt[:, q0:512],
                                        bass_ap_3d(vt_all, voff, 528, 2, 66),
                                        eap, start=(pp == 0), stop=False,
                                        perf_mode=DR)
                                    nc.tensor.matmul(
                                        zt[:, q0:512],
                                        bass_ap_3d(rt_all, voff, 528, 2, 66),
                                        eap, start=False,
                                        stop=(pp == npairs - 1),
                                        perf_mode=DR)
                        et2 = None
                        for kt in range(ktiles):
                            sub = kt % 2
                            if sub == 0:
                                et2 = etp.tile([128, 2048], f8, tag="et",
                                               name="et")
                            base = sub * 1024
                            st2 = stp.tile([128, 1024], f32, tag="st2",
                                           name="st2")
                            j = kt - 4 * qc
                            # left-of-diagonal q-columns are fully masked:
                            # narrow the score matmuls to the valid range
                            q0 = max(j, 0) * 128
                            nc.tensor.matmul(
                                st2[:, q0:512],
                                kt_all[0:64, hp * S + kt * 128:
                                       hp * S + (kt + 1) * 128],
                                qt_all[0:64, hp * S + qc * 512 + q0:
                                       hp * S + (qc + 1) * 512],
                                start=True, stop=True, tile_position=(0, 0))
                            nc.tensor.matmul(
                                st2[:, 512 + q0:1024],
                                kt_all[64:128, hp * S + kt * 128:
                                       hp * S + (kt + 1) * 128],
                                qt_all[64:128, hp * S + qc * 512 + q0:
                                       hp * S + (qc + 1) * 512],
                                start=True, stop=True, tile_position=(64, 0))
                            if j > 0:
                                # left-of-diagonal q-subtiles fully masked:
                                # skip their exp. The ZT matmuls are narrowed
                                # to cols >= (pair's first j)*128, so only the
                                # odd member of each kt-pair needs a 128-col
                                # zero strip at its j-1 boundary
                                if sub == 1:
                                    nc.gpsimd.memset(
                                        bass_ap_3d(et2, base + (j - 1) * 128,
                                                   512, 2, 128), 0.0)
                                nc.scalar.activation(
                                    bass_ap_3d(et2, base + j * 128, 512, 2,
                                               512 - j * 128),
                                    bass_ap_3d(st2, j * 128, 512, 2,
                                               512 - j * 128),
                                    AF.Exp, scale=0.125)
                            else:
                                nc.scalar.activation(
                                    bass_ap_3d(et2, base, 512, 2, 512),
                                    st2[:], AF.Exp, scale=0.125)
                            if j >= 0:
                                # causal mask on the diagonal 128-block of ET
                                for half in range(2):
                                    blk = et2[:, base + half * 512 + j * 128:
                                              base + half * 512 + (j + 1) * 128]
                                    nc.gpsimd.affine_select(
                                        out=blk, in_=blk, compare_op=ALU.is_ge,
                                        fill=0.0, base=0, pattern=[[1, 128]],
                                        channel_multiplier=-1)
                            if sub == 1:
                                pending.append((kt // 2, et2))
                                flush_zt(3)
                        flush_zt(0)
                        nc.vector.tensor_copy(
                            ztall[:, (2 * hp) * 512:(2 * hp + 1) * 512],
                            zt0[0:65, :])
                        nc.vector.tensor_copy(
                            ztall[:, (2 * hp + 1) * 512:(2 * hp + 2) * 512],
                            zt1[0:65, :])
                        for fn in hooks.get(hp, []):
                            fn()

                def tail_proj(qc):
                    zsr = zrp.tile([128, 512], f16, tag="zsr")
                    nc.vector.tensor_copy(zsr[0:DH, :],
                                          zsum[:, qc * 512:(qc + 1) * 512])
                    nc.gpsimd.dma_start(zsr[DH:2 * DH, :],
                                        zsum[:, qc * 512:(qc + 1) * 512])
                    for qp in range(2):
                        for nn in range(2):
                            for sub2 in range(2):
                                po = ppp.tile([128, 512], f32, tag="pp",
                                              name="po")
                                if sub2 == 0:
                                    nc.tensor.matmul(
                                        po[:],
                                        zsr[0:DH,
                                            (2 * qp) * 128:(2 * qp + 1) * 128],
                                        wo_sb[0:DH, nn * 512:(nn + 1) * 512],
                                        start=True, stop=True,
                                        tile_position=(0, 0))
                                else:
                                    nc.tensor.matmul(
                                        po[:],
                                        zsr[DH:128, (2 * qp + 1) * 128:
                                            (2 * qp + 2) * 128],
                                        wo_sb[DH:128, nn * 512:(nn + 1) * 512],
                                        start=True, stop=True,
                                        tile_position=(64, 0))
                                osb = osbp.tile([128, 512], f32, tag="osb")
                                nc.vector.tensor_copy(osb[:], po[:])
                                r0 = (qc * 512 + (2 * qp) * 128
                                      + sub2 * 128)
                                nc.sync.dma_start(
                                    out[r0:r0 + 128,
                                        nn * 512:(nn + 1) * 512],
                                    osb[:])

                def make_tail(qc):
                    def hook_for(hp):
                        def hook():
                            ztall = ztalls[qc]
                            if hp == 3:
                                ztalls.pop(qc)
                            rld = rldp.tile([1, 1024], f32, tag="rld",
                                            name=f"rld{qc}_{hp}")
                            lbs = lbsp.tile([DH, 1024], f32, tag="lbs",
                                            name=f"lbs{qc}_{hp}")
                            c0 = (2 * hp) * 512
                            cols = slice(c0, c0 + 1024)
                            nc.sync.dma_start(rld[:], ztall[64:65, cols])
                            nc.sync.dma_start(
                                lbs[:], _bcast_ap(rld, 0, 0, DH, 1024))
                            # reciprocal AFTER the broadcast: the custom DVE
                            # op misreads inputs at a nonzero partition base,
                            # and lbs sits at partition 0
                            lbi = lbsp.tile([DH, 1024], f32, tag="lbi",
                                            name=f"lbi{qc}_{hp}")
                            nc.vector.reciprocal_approx_fast(
                                out=lbi[:], in_=lbs[:])
                            for hh in (2 * hp, 2 * hp + 1):
                                hcols = slice(hh * 512, (hh + 1) * 512)
                                lcols = slice((hh - 2 * hp) * 512,
                                              (hh - 2 * hp + 1) * 512)
                                if hh == 0:
                                    nc.vector.tensor_tensor(
                                        zsum[:, qc * 512:(qc + 1) * 512],
                                        ztall[0:DH, hcols], lbi[:, lcols],
                                        op=ALU.mult)
                                else:
                                    zn = znp.tile([DH, 512], f32, tag="zn")
                                    nc.vector.tensor_tensor(
                                        zn[:], ztall[0:DH, hcols],
                                        lbi[:, lcols], op=ALU.mult)
                                    nc.vector.tensor_tensor(
                                        zsum[:, qc * 512:(qc + 1) * 512],
                                        zsum[:, qc * 512:(qc + 1) * 512],
                                        zn[:], op=ALU.add)
                        return hook

                    return hook_for, (lambda: tail_proj(qc))

                tail_fns = {}
                for nq in range(NQ):
                    if nq % 2 == 0:
                        proj_pair(nq // 2)
                    hooks = {i: [] for i in range(4)}
                    if nq > 0:
                        th, tp = tail_fns.pop(nq - 1)
                        if nq < NQ - 1:
                            for i in range(4):
                                hooks[i].append(th(i))
                            hooks[3].append(tp)
                        else:
                            hooks[0] += [th(0), th(1)]
                            hooks[1] += [th(2), th(3), tp]
                    if nq == NQ - 1:
                        th3, tp3 = make_tail(nq)
                        for i in range(4):
                            hooks[i].append(th3(i))
                        hooks[3].append(tp3)
                    else:
                        tail_fns[nq] = make_tail(nq)
                    attention(nq, hooks)
    nc.compile()
    return nc


def kernel(**inputs):
    x = np.asarray(inputs["x"], dtype=np.float32)
    WQ = np.asarray(inputs["WQ"], dtype=np.float32)
    bQ = np.asarray(inputs["bQ"], dtype=np.float32)
    WK = np.asarray(inputs["WK"], dtype=np.float32)
    bK = np.asarray(inputs["bK"], dtype=np.float32)
    WV = np.asarray(inputs["WV"], dtype=np.float32)
    bV = np.asarray(inputs["bV"], dtype=np.float32)
    WO = np.asarray(inputs["WO"], dtype=np.float32)
    bO = np.asarray(inputs["bO"], dtype=np.float32)

    from concourse.bass_utils import run_bass_kernel_spmd

    if "nc" not in _prog:
        _prog["nc"] = _build()
    nc = _prog["nc"]

    in_maps = []
    for c in range(NCORES):
        b, g = c // 2, c % 2
        sl = slice(g * GD, (g + 1) * GD)
        in_maps.append({
            "x": np.ascontiguousarray(x[b]).astype(np.float16),
            "wq": np.ascontiguousarray(WQ[:, sl]).astype(np.float16),
            "wk": np.ascontiguousarray(WK[:, sl]).astype(np.float16),
            "wv": np.ascontiguousarray(WV[:, sl]).astype(np.float16),
            "bq": np.ascontiguousarray(bQ[sl]).reshape(1, GD).astype(np.float16),
            "bk": np.ascontiguousarray(bK[sl]).reshape(1, GD).astype(np.float16),
            "wo": WO.astype(np.float16),
        })
    _prog["in_maps"] = in_maps
    globals()["_last_in_maps"] = in_maps
    res = run_bass_kernel_spmd(nc, in_maps, core_ids=list(range(NCORES)))
    _prog["res"] = res
    parts = [r["out"] for r in res.results]

    extra = bV.reshape(H, DH).sum(0) @ WO + np.float32(H) * bO
    out = np.empty((B, S, D), dtype=np.float32)
    for b in range(B):
        out[b] = parts[2 * b] + parts[2 * b + 1] + extra
    return out



# revision 7
# speedup vs baseline: 1.0464x; 1.0464x over previous
"""Causal self-attention kernel for 8 Trainium2 NeuronCores.

Sharding: core c -> (batch b = c//2, head-group g = c%2). Each core computes
the attention output contribution of 8 heads for one batch element:
    P_c = (sum_{h in group} softmax(Q_h K_h^T / 8 + causal) V_h) @ WO
Host epilogue: out[b] = P_{2b} + P_{2b+1} + (sum_h bV_h) @ WO + 16*bO
(the V-bias commutes through softmax normalization: softmax rows sum to 1;
the K-bias cancels entirely: softmax((Q+bq)(K+bk)^T) = softmax((Q+bq)K^T)
because Q.bk is constant along the key axis.)

v4 design notes (evolved from v3, 339.7us):
  - Projections and scores fp16; ET fp8e4m3; V as fp8 V8 + fp8 residual R8
    consumed by DoubleRow A@V (unchanged math from v3).
  - K projection carries NO bias -> evicted by ScalarE (activation Copy);
    V8 cast f32->fp8 also on ScalarE; both run in proj phases where the
    scalar engine is otherwise idle. DVE keeps Q bias add + R8 residual.
  - Normalization fused into PSUM eviction: copy the l-row to SBUF, PE-
    broadcast it with a ones[1,64] stationary matmul into a PSUM tile,
    reciprocal_approx_fast, then tensor_tensor multiply zt (PSUM) straight
    into zsum.  The v3 ztall intermediate, its 32 DVE copies, and the
    DRAM-bounce partition broadcast are gone.
  - Cross-head-pair software pipelining: the last AV pairs + the norm chain
    of head-pair hp issue as a deferred "carry" inside hp+1's score stream
    (after 3 score tiles), removing the per-hp PE drain bubble.
  - Projection work is sliced into ~1us filler units; attention(qc)
    consumes units of chunk qc+1's projection at hp boundaries, the rest
    issue between chunks. Keeps the PE fed where ScalarE exp lags.
  - x loaded as column-half whole-chunk DMAs split across the sync and
    vector queues, all issued upfront (xs pool holds all 4 chunks).
  - Output is f16 (host upcasts): halves the out DMA; out DMAs ride the
    gpsimd queue which is idle after the weight loads.
"""
import numpy as np

B, S, D, H, DH = 4, 2048, 1024, 16, 64
HPC = 8            # heads per core
GD = HPC * DH      # 512 = group width
NCORES = 8
NQ = S // 512      # 4 q/s chunks of 512
NKT = S // 128     # 16 k-tiles
NDT = D // 128     # 8 d-tiles

_prog = {}


def bass_ap_3d(tile_t, offset, stride, n, inner):
    """AP view [128p, n, inner] over a tile's free dim: col = offset + i*stride + c."""
    import concourse.bass as bass
    ap = tile_t[:]
    return bass.AP(ap.tensor, ap.offset + offset,
                   [ap.ap[0], [stride, n], [1, inner]])


def _build():
    import concourse.bacc as bacc
    import concourse.tile as tile
    from concourse import mybir
    import concourse.bass as bass

    f32 = mybir.dt.float32
    f16 = mybir.dt.float16
    f8 = mybir.dt.float8e4
    AF = mybir.ActivationFunctionType
    ALU = mybir.AluOpType
    DR = mybir.MatmulPerfMode.DoubleRow

    nc = bacc.Bacc(None, target_bir_lowering=False, debug=False)
    x = nc.dram_tensor("x", [S, D], f16, kind="ExternalInput")
    wq = nc.dram_tensor("wq", [D, GD], f16, kind="ExternalInput")
    wk = nc.dram_tensor("wk", [D, GD], f16, kind="ExternalInput")
    wv = nc.dram_tensor("wv", [D, GD], f16, kind="ExternalInput")
    bq = nc.dram_tensor("bq", [1, GD], f16, kind="ExternalInput")
    wo = nc.dram_tensor("wo", [DH, D], f16, kind="ExternalInput")
    out = nc.dram_tensor("out", [S, D], f16, kind="ExternalOutput")

    with tile.TileContext(nc) as tc:
        with tc.tile_pool(name="const", bufs=1) as constp, \
             tc.tile_pool(name="big", bufs=1) as bigp:
            # ---- persistent tensors ----
            xs_all = bigp.tile([128, NQ * 4096], f16, tag="xs")   # chunk c at c*4096
            xt_all = bigp.tile([128, NDT * S], f16, tag="xt")     # d-tile j at j*S
            qt_all = bigp.tile([128, 4 * S], f16, tag="qt")       # m-tile m at m*S
            kt_all = bigp.tile([128, 4 * S], f16, tag="kt")
            vt_all = bigp.tile([128, NKT * 528], f8, tag="vt")    # V8 + ones col
            rt_all = bigp.tile([128, NKT * 528], f8, tag="rt")    # fp8 residual
            zsum = bigp.tile([DH, S], f32, tag="zsum")

            idt = constp.tile([128, 128], f16, tag="idt")
            bq_t = constp.tile([128, 4], f32, tag="bq_t")
            wo_sb = constp.tile([128, D], f16, tag="wo_sb")
            ones_sb = constp.tile([1, DH], f16, tag="ones_sb")

            # ---- input DMAs: x chunks split into column halves across two
            # queues so the first transposes can start after ~1.5us ----
            for c in range(NQ):
                for half, eng in ((0, nc.sync), (1, nc.scalar)):
                    dst = bass.AP(xs_all[:].tensor,
                                  xs_all[:].offset + c * 4096 + half * 512,
                                  [xs_all[:].ap[0], [1024, 4], [1, 512]])
                    src = bass.AP(x, c * 4 * 131072 + half * 512,
                                  [[1024, 128], [131072, 4], [1, 512]])
                    eng.dma_start(dst, src)
            # gpsimd stream: identity + small consts first (transposes need
            # idt early), then the big weight DMAs
            from concourse.masks import make_identity
            make_identity(nc, idt[:])
            nc.gpsimd.memset(ones_sb[:], 1.0)
            nc.gpsimd.dma_start(bq_t[:], bass.AP(bq, 0, [[1, 128], [128, 4]]))

            with tc.tile_pool(name="wts", bufs=1) as wtp, \
                 tc.tile_pool(name="et", bufs=6) as etp, \
                 tc.tile_pool(name="lrow", bufs=2) as lrp, \
                 tc.tile_pool(name="lbi", bufs=2) as lbip, \
                 tc.tile_pool(name="zn", bufs=2) as znp, \
                 tc.tile_pool(name="zr", bufs=2) as zrp, \
                 tc.tile_pool(name="osb", bufs=3) as osbp, \
                 tc.tile_pool(name="stp", bufs=2, space="PSUM") as stp, \
                 tc.tile_pool(name="ppp", bufs=2, space="PSUM") as ppp, \
                 tc.tile_pool(name="ztp", bufs=2, space="PSUM") as ztp:
                wq_all = wtp.tile([128, NDT * GD], f16, tag="wq_all")
                wk_all = wtp.tile([128, NDT * GD], f16, tag="wk_all")
                wv_all = wtp.tile([128, NDT * GD], f16, tag="wv_all")
                for (w_all, w_dram) in ((wq_all, wq), (wk_all, wk),
                                        (wv_all, wv)):
                    nc.gpsimd.dma_start(
                        bass_ap_3d(w_all, 0, GD, NDT, GD),
                        bass.AP(w_dram, 0, [[GD, 128], [128 * GD, NDT],
                                            [1, GD]]))
                # ones/zero pad columns of vt/rt for ALL k-tiles, once
                nc.gpsimd.memset(
                    bass_ap_3d(vt_all, DH, 66, NKT * HPC, 2), 0.0)
                nc.gpsimd.memset(
                    bass_ap_3d(vt_all, DH, 66, NKT * HPC, 1), 1.0)
                nc.gpsimd.memset(
                    bass_ap_3d(rt_all, DH, 66, NKT * HPC, 2), 0.0)
                nc.gpsimd.dma_start(wo_sb[0:DH, :], wo[:])
                nc.gpsimd.dma_start(wo_sb[DH:2 * DH, :], wo[:])

                # ---------- projection filler units ----------
                def transpose_unit(c, jj):
                    # transposes d-tiles 2jj, 2jj+1 of chunk c
                    pt = ppp.tile([128, 1024], f16, tag="pp", name="pt")
                    for j2 in range(2):
                        j = jj * 2 + j2
                        for st4 in range(4):
                            col = c * 4096 + st4 * 1024 + j * 128
                            nc.tensor.transpose(
                                pt[:, j2 * 512 + st4 * 128:
                                   j2 * 512 + (st4 + 1) * 128],
                                xs_all[:, col:col + 128],
                                idt[:])
                    dst = bass_ap_3d(xt_all, (jj * 2) * S + c * 512, S, 2, 512)
                    nc.vector.tensor_copy(dst, bass_ap_3d(pt, 0, 512, 2, 512))

                def qkproj_unit(c, which, m):
                    # one m-tile (2 heads) of the Q or K projection of chunk c
                    w_all = wq_all if which == 0 else wk_all
                    dest = qt_all if which == 0 else kt_all
                    ps = ppp.tile([128, 512], f32, tag="pp", name="ps")
                    for k in range(NDT):
                        nc.tensor.matmul(
                            ps[:],
                            w_all[:, k * GD + m * 128: k * GD + (m + 1) * 128],
                            xt_all[:, k * S + c * 512: k * S + (c + 1) * 512],
                            start=(k == 0), stop=(k == NDT - 1))
                    dcols = dest[:, m * S + c * 512: m * S + (c + 1) * 512]
                    if which == 0:
                        nc.vector.tensor_scalar_add(dcols, ps[:],
                                                    bq_t[:, m:m + 1])
                    else:
                        nc.scalar.activation(dcols, ps[:], AF.Copy)

                def vproj_unit(c, st4):
                    st = c * 4 + st4
                    ps = ppp.tile([128, 512], f32, tag="pp", name="ps")
                    for k in range(NDT):
                        nc.tensor.matmul(
                            ps[:],
                            xt_all[:, k * S + st * 128: k * S + (st + 1) * 128],
                            wv_all[:, k * GD:(k + 1) * GD],
                            start=(k == 0), stop=(k == NDT - 1))
                    dst = bass_ap_3d(vt_all, st * 528, 66, HPC, DH)
                    srcap = bass_ap_3d(ps, 0, DH, HPC, DH)
                    nc.scalar.activation(dst, srcap, AF.Copy)
                    rdst = bass_ap_3d(rt_all, st * 528, 66, HPC, DH)
                    nc.vector.tensor_tensor(rdst, srcap, dst, op=ALU.subtract)

                def proj_units(c):
                    units = []
                    for jj in range(4):
                        units.append(lambda c=c, jj=jj: transpose_unit(c, jj))
                    for m in range(4):
                        for which in range(2):
                            units.append(lambda c=c, w=which, m=m:
                                         qkproj_unit(c, w, m))
                    for st4 in range(4):
                        units.append(lambda c=c, s=st4: vproj_unit(c, s))
                    return units

                # ---------- attention ----------
                def make_drain(qc, hp, zt0, zt1, pending, npairs):
                    """Deferred: last AVs of (qc,hp), then fused norm."""
                    def av(pp, pet):
                        jz = max(2 * pp - 4 * qc, 0)
                        q0 = jz * 128
                        for half, zt in ((0, zt0), (1, zt1)):
                            eap = bass_ap_3d(pet, half * 512 + q0,
                                             1024, 2, 512 - q0)
                            voff = (2 * pp) * 528 + (2 * hp + half) * 66
                            nc.tensor.matmul(
                                zt[:, q0:512],
                                bass_ap_3d(vt_all, voff, 528, 2, 66),
                                eap, start=(pp == 0), stop=False,
                                perf_mode=DR)
                            nc.tensor.matmul(
                                zt[:, q0:512],
                                bass_ap_3d(rt_all, voff, 528, 2, 66),
                                eap, start=False,
                                stop=(pp == npairs - 1),
                                perf_mode=DR)

                    def drain():
                        while pending:
                            av(*pending.pop(0))
                        # fused normalization: l-row -> SBUF, PE broadcast,
                        # reciprocal, multiply into zsum
                        lrow = lrp.tile([1, 1024], f16, tag="lrow")
                        nc.vector.tensor_copy(lrow[:, 0:512], zt0[64:65, :])
                        nc.vector.tensor_copy(lrow[:, 512:1024], zt1[64:65, :])
                        zcols = slice(qc * 512, (qc + 1) * 512)
                        for half, zt in ((0, zt0), (1, zt1)):
                            lb = ppp.tile([DH, 512], f32, tag="pp", name="lb")
                            nc.tensor.matmul(
                                lb[:], ones_sb[:],
                                lrow[:, half * 512:(half + 1) * 512],
                                start=True, stop=True)
                            lbi = lbip.tile([DH, 512], f32, tag="lbi")
                            nc.vector.reciprocal_approx_fast(
                                out=lbi[:], in_=lb[:])
                            if 2 * hp + half == 0:
                                nc.vector.tensor_tensor(
                                    zsum[:, zcols], zt[0:DH, :], lbi[:],
                                    op=ALU.mult)
                            else:
                                zn = znp.tile([DH, 512], f32, tag="zn")
                                nc.vector.tensor_tensor(
                                    zn[:], zt[0:DH, :], lbi[:], op=ALU.mult)
                                nc.vector.tensor_tensor(
                                    zsum[:, zcols], zsum[:, zcols], zn[:],
                                    op=ALU.add)
                    return drain

                def attention(qc, carry, filler, posts):
                    """carry: deferred drain from the previous (qc,hp);
                    filler: proj units to interleave; posts: deferred
                    tail-projection units of the previous chunk."""
                    ktiles = 4 * qc + 4
                    npairs = ktiles // 2
                    for hp in range(4):
                        zt0 = ztp.tile([66, 512], f32, tag="zt", name="zt0")
                        zt1 = ztp.tile([66, 512], f32, tag="zt", name="zt1")
                        pending = []

                        def av_flush(lag):
                            while len(pending) > lag:
                                pp, pet = pending.pop(0)
                                jz = max(2 * pp - 4 * qc, 0)
                                q0 = jz * 128
                                for half, zt in ((0, zt0), (1, zt1)):
                                    eap = bass_ap_3d(pet, half * 512 + q0,
                                                     1024, 2, 512 - q0)
                                    voff = ((2 * pp) * 528
                                            + (2 * hp + half) * 66)
                                    nc.tensor.matmul(
                                        zt[:, q0:512],
                                        bass_ap_3d(vt_all, voff, 528, 2, 66),
                                        eap, start=(pp == 0), stop=False,
                                        perf_mode=DR)
                                    nc.tensor.matmul(
                                        zt[:, q0:512],
                                        bass_ap_3d(rt_all, voff, 528, 2, 66),
                                        eap, start=False,
                                        stop=(pp == npairs - 1),
                                        perf_mode=DR)

                        et2 = None
                        for kt in range(ktiles):
                            sub = kt % 2
                            if sub == 0:
                                et2 = etp.tile([128, 2048], f8, tag="et",
                                               name="et")
                            base = sub * 1024
                            st2 = stp.tile([128, 1024], f32, tag="st2",
                                           name="st2")
                            j = kt - 4 * qc
                            q0 = max(j, 0) * 128
                            nc.tensor.matmul(
                                st2[:, q0:512],
                                kt_all[0:64, hp * S + kt * 128:
                                       hp * S + (kt + 1) * 128],
                                qt_all[0:64, hp * S + qc * 512 + q0:
                                       hp * S + (qc + 1) * 512],
                                start=True, stop=True, tile_position=(0, 0))
                            nc.tensor.matmul(
                                st2[:, 512 + q0:1024],
                                kt_all[64:128, hp * S + kt * 128:
                                       hp * S + (kt + 1) * 128],
                                qt_all[64:128, hp * S + qc * 512 + q0:
                                       hp * S + (qc + 1) * 512],
                                start=True, stop=True, tile_position=(64, 0))
                            if j > 0:
                                if sub == 1:
                                    nc.gpsimd.memset(
                                        bass_ap_3d(et2, base + (j - 1) * 128,
                                                   512, 2, 128), 0.0)
                                nc.scalar.activation(
                                    bass_ap_3d(et2, base + j * 128, 512, 2,
                                               512 - j * 128),
                                    bass_ap_3d(st2, j * 128, 512, 2,
                                               512 - j * 128),
                                    AF.Exp, scale=0.125)
                            else:
                                nc.scalar.activation(
                                    bass_ap_3d(et2, base, 512, 2, 512),
                                    st2[:], AF.Exp, scale=0.125)
                            if j >= 0:
                                for half in range(2):
                                    blk = et2[:, base + half * 512 + j * 128:
                                              base + half * 512 + (j + 1) * 128]
                                    nc.gpsimd.affine_select(
                                        out=blk, in_=blk, compare_op=ALU.is_ge,
                                        fill=0.0, base=0, pattern=[[1, 128]],
                                        channel_multiplier=-1)
                            if sub == 1:
                                pending.append((kt // 2, et2))
                            if kt == 2:
                                if carry[0] is not None:
                                    carry[0]()
                                    carry[0] = None
                                if posts:
                                    posts.pop(0)()
                            elif kt == 3 and filler:
                                filler.pop(0)()
                            elif kt == 5 and posts:
                                posts.pop(0)()
                            if sub == 1:
                                av_flush(3)
                        av_flush(2)
                        carry[0] = make_drain(qc, hp, zt0, zt1, pending,
                                              npairs)

                def tail_units(qc):
                    """Out-projection of chunk qc as 4 deferred units."""
                    zsr = zrp.tile([128, 512], f16, tag="zsr",
                                   name=f"zsr{qc}")
                    state = {"prepped": False}

                    def prep():
                        nc.vector.tensor_copy(zsr[0:DH, :],
                                              zsum[:, qc * 512:(qc + 1) * 512])
                        nc.gpsimd.dma_start(
                            zsr[DH:2 * DH, :],
                            zsum[:, qc * 512:(qc + 1) * 512])

                    def unit(qp):
                        if not state["prepped"]:
                            prep()
                            state["prepped"] = True
                        for sub2 in range(2):
                            for nn in range(2):
                                po = ppp.tile([128, 512], f32, tag="pp",
                                              name="po")
                                if sub2 == 0:
                                    nc.tensor.matmul(
                                        po[:],
                                        zsr[0:DH,
                                            (2 * qp) * 128:(2 * qp + 1) * 128],
                                        wo_sb[0:DH, nn * 512:(nn + 1) * 512],
                                        start=True, stop=True,
                                        tile_position=(0, 0))
                                else:
                                    nc.tensor.matmul(
                                        po[:],
                                        zsr[DH:128, (2 * qp + 1) * 128:
                                            (2 * qp + 2) * 128],
                                        wo_sb[DH:128, nn * 512:(nn + 1) * 512],
                                        start=True, stop=True,
                                        tile_position=(64, 0))
                                osb = osbp.tile([128, 512], f16, tag="osb")
                                nc.vector.tensor_copy(osb[:], po[:])
                                r0 = qc * 512 + (2 * qp) * 128 + sub2 * 128
                                nc.gpsimd.dma_start(
                                    out[r0:r0 + 128,
                                        nn * 512:(nn + 1) * 512],
                                    osb[:])
                    return [lambda qp=qp: unit(qp) for qp in range(2)]

                # ---------- main schedule ----------
                carry = [None]
                filler = []
                posts = []
                for u in proj_units(0):
                    u()
                for qc in range(NQ):
                    if qc + 1 < NQ:
                        filler.extend(proj_units(qc + 1))
                    attention(qc, carry, filler, posts)
                    # drain leftover proj filler between chunks
                    for u in filler:
                        u()
                    filler = []
                    if qc > 0:
                        # any tail units of chunk qc-1 not yet consumed
                        for u in posts:
                            u()
                        posts = []
                    posts.extend(tail_units(qc))
                # final chunk: drain + its tail immediately
                carry[0]()
                carry[0] = None
                for u in posts:
                    u()
    nc.compile()
    return nc


def kernel(**inputs):
    x = np.asarray(inputs["x"], dtype=np.float32)
    WQ = np.asarray(inputs["WQ"], dtype=np.float32)
    bQ = np.asarray(inputs["bQ"], dtype=np.float32)
    WK = np.asarray(inputs["WK"], dtype=np.float32)
    WV = np.asarray(inputs["WV"], dtype=np.float32)
    bV = np.asarray(inputs["bV"], dtype=np.float32)
    WO = np.asarray(inputs["WO"], dtype=np.float32)
    bO = np.asarray(inputs["bO"], dtype=np.float32)

    from concourse.bass_utils import run_bass_kernel_spmd

    if "nc" not in _prog:
        _prog["nc"] = _build()
    nc = _prog["nc"]

    in_maps = []
    for c in range(NCORES):
        b, g = c // 2, c % 2
        sl = slice(g * GD, (g + 1) * GD)
        in_maps.append({
            "x": np.ascontiguousarray(x[b]).astype(np.float16),
            "wq": np.ascontiguousarray(WQ[:, sl]).astype(np.float16),
            "wk": np.ascontiguousarray(WK[:, sl]).astype(np.float16),
            "wv": np.ascontiguousarray(WV[:, sl]).astype(np.float16),
            "bq": np.ascontiguousarray(bQ[sl]).reshape(1, GD).astype(np.float16),
            "wo": WO.astype(np.float16),
        })
    _prog["in_maps"] = in_maps
    globals()["_last_in_maps"] = in_maps
    res = run_bass_kernel_spmd(nc, in_maps, core_ids=list(range(NCORES)))
    _prog["res"] = res
    parts = [r["out"].astype(np.float32) for r in res.results]

    extra = bV.reshape(H, DH).sum(0) @ WO + np.float32(H) * bO
    out = np.empty((B, S, D), dtype=np.float32)
    for b in range(B):
        out[b] = parts[2 * b] + parts[2 * b + 1] + extra
    return out


# revision 18
# speedup vs baseline: 1.1009x; 1.0521x over previous
"""Causal self-attention kernel for 8 Trainium2 NeuronCores.

Sharding: core c -> (batch b = c//2, head-group g = c%2). Each core computes
the attention output contribution of 8 heads for one batch element:
    P_c = (sum_{h in group} softmax(Q_h K_h^T / 8 + causal) V_h) @ WO
Host epilogue: out[b] = P_{2b} + P_{2b+1} + (sum_h bV_h) @ WO + 16*bO
(the V-bias commutes through softmax normalization: softmax rows sum to 1;
the K-bias cancels entirely: softmax((Q+bq)(K+bk)^T) = softmax((Q+bq)K^T)
because Q.bk is constant along the key axis.)

v4 design notes (evolved from v3, 339.7us):
  - Projections and scores fp16; ET fp8e4m3; V as fp8 V8 + fp8 residual R8
    consumed by DoubleRow A@V (unchanged math from v3).
  - K projection carries NO bias -> evicted by ScalarE (activation Copy);
    V8 cast f32->fp8 also on ScalarE; both run in proj phases where the
    scalar engine is otherwise idle. DVE keeps Q bias add + R8 residual.
  - Normalization fused into PSUM eviction: copy the l-row to SBUF, PE-
    broadcast it with a ones[1,64] stationary matmul into a PSUM tile,
    reciprocal_approx_fast, then tensor_tensor multiply zt (PSUM) straight
    into zsum.  The v3 ztall intermediate, its 32 DVE copies, and the
    DRAM-bounce partition broadcast are gone.
  - Cross-head-pair software pipelining: the last AV pairs + the norm chain
    of head-pair hp issue as a deferred "carry" inside hp+1's score stream
    (after 3 score tiles), removing the per-hp PE drain bubble.
  - Projection work is sliced into ~1us filler units; attention(qc)
    consumes units of chunk qc+1's projection at hp boundaries, the rest
    issue between chunks. Keeps the PE fed where ScalarE exp lags.
  - x loaded as column-half whole-chunk DMAs split across the sync and
    vector queues, all issued upfront (xs pool holds all 4 chunks).
  - Output is f16 (host upcasts): halves the out DMA; out DMAs ride the
    gpsimd queue which is idle after the weight loads.
"""
import numpy as np

B, S, D, H, DH = 4, 2048, 1024, 16, 64
HPC = 8            # heads per core
GD = HPC * DH      # 512 = group width
NCORES = 8
NQ = S // 512      # 4 q/s chunks of 512
NKT = S // 128     # 16 k-tiles
NDT = D // 128     # 8 d-tiles

_prog = {}


def bass_ap_3d(tile_t, offset, stride, n, inner):
    """AP view [128p, n, inner] over a tile's free dim: col = offset + i*stride + c."""
    import concourse.bass as bass
    ap = tile_t[:]
    return bass.AP(ap.tensor, ap.offset + offset,
                   [ap.ap[0], [stride, n], [1, inner]])


def _build():
    import concourse.bacc as bacc
    import concourse.tile as tile
    from concourse import mybir
    import concourse.bass as bass

    f32 = mybir.dt.float32
    f16 = mybir.dt.float16
    f8 = mybir.dt.float8e4
    AF = mybir.ActivationFunctionType
    ALU = mybir.AluOpType
    DR = mybir.MatmulPerfMode.DoubleRow

    nc = bacc.Bacc(None, target_bir_lowering=False, debug=False)
    x = nc.dram_tensor("x", [S, D], f16, kind="ExternalInput")
    wq = nc.dram_tensor("wq", [D, GD], f16, kind="ExternalInput")
    wk = nc.dram_tensor("wk", [D, GD], f16, kind="ExternalInput")
    wv = nc.dram_tensor("wv", [D, GD], f16, kind="ExternalInput")
    bq = nc.dram_tensor("bq", [1, GD], f16, kind="ExternalInput")
    wo = nc.dram_tensor("wo", [DH, D], f16, kind="ExternalInput")
    out = nc.dram_tensor("out", [S, D], f16, kind="ExternalOutput")

    with tile.TileContext(nc) as tc:
        with tc.tile_pool(name="const", bufs=1) as constp, \
             tc.tile_pool(name="big", bufs=1) as bigp:
            # ---- persistent tensors ----
            xs_all = bigp.tile([128, NQ * 4096], f16, tag="xs")   # chunk c at c*4096
            xt_all = bigp.tile([128, NDT * S], f16, tag="xt")     # d-tile j at j*S
            qt_all = bigp.tile([128, 4 * S], f16, tag="qt")       # m-tile m at m*S
            kt_all = bigp.tile([128, 4 * S], f16, tag="kt")
            vt_all = bigp.tile([128, NKT * 528], f8, tag="vt")    # ones+V8 cols
            rt_all = bigp.tile([128, NKT * 528], f8, tag="rt")    # fp8 residual
            # rows 1-64 hold sum_h Z_h/l_h (row 0 unused: keeps partition
            # alignment with zt, whose row 0 is the l accumulator)
            zsum = bigp.tile([DH + 1, S], f32, tag="zsum")

            idt = constp.tile([128, 128], f16, tag="idt")
            bq_t = constp.tile([128, 4], f32, tag="bq_t")
            wo_sb = constp.tile([128, D], f16, tag="wo_sb")

            # ---- input DMAs: x chunk 0 split into column halves across the
            # sync and scalar queues so the first transposes start ~1.5us in;
            # chunks 1-3 follow the weights on the gpsimd queue (needed much
            # later, keeps HBM clear for the weights) ----
            def x_chunk_dma(c, half, eng):
                dst = bass.AP(xs_all[:].tensor,
                              xs_all[:].offset + c * 4096 + half * 512,
                              [xs_all[:].ap[0], [1024, 4], [1, 512]])
                src = bass.AP(x, c * 4 * 131072 + half * 512,
                              [[1024, 128], [131072, 4], [1, 512]])
                eng.dma_start(dst, src)
            x_chunk_dma(0, 0, nc.sync)
            x_chunk_dma(0, 1, nc.scalar)
            # gpsimd stream: identity + small consts first (transposes need
            # idt early), then the big weight DMAs
            from concourse.masks import make_identity
            make_identity(nc, idt[:])
            nc.gpsimd.dma_start(bq_t[:], bass.AP(bq, 0, [[1, 128], [128, 4]]))

            with tc.tile_pool(name="wts", bufs=1) as wtp, \
                 tc.tile_pool(name="et", bufs=6) as etp, \
                 tc.tile_pool(name="lrow", bufs=2) as lrp, \
                 tc.tile_pool(name="lbi", bufs=2) as lbip, \
                 tc.tile_pool(name="zn", bufs=2) as znp, \
                 tc.tile_pool(name="zr", bufs=2) as zrp, \
                 tc.tile_pool(name="osb", bufs=3) as osbp, \
                 tc.tile_pool(name="stp", bufs=2, space="PSUM") as stp, \
                 tc.tile_pool(name="ppp", bufs=2, space="PSUM") as ppp, \
                 tc.tile_pool(name="ztp", bufs=2, space="PSUM") as ztp:
                wq_all = wtp.tile([128, NDT * GD], f16, tag="wq_all")
                wk_all = wtp.tile([128, NDT * GD], f16, tag="wk_all")
                wv_all = wtp.tile([128, NDT * GD], f16, tag="wv_all")
                for (w_all, w_dram) in ((wq_all, wq), (wk_all, wk),
                                        (wv_all, wv)):
                    nc.gpsimd.dma_start(
                        bass_ap_3d(w_all, 0, GD, NDT, GD),
                        bass.AP(w_dram, 0, [[GD, 128], [128 * GD, NDT],
                                            [1, GD]]))
                # vt/rt layout per (kt, head): col 0 = ones (the l
                # accumulator lands at PSUM partition 0 so the reciprocal
                # can read it directly), cols 1-64 = V8, col 65 = pad.
                nc.gpsimd.memset(
                    bass_ap_3d(vt_all, 0, 66, NKT * HPC, 1), 1.0)
                nc.gpsimd.memset(
                    bass_ap_3d(vt_all, 65, 66, NKT * HPC, 1), 0.0)
                nc.gpsimd.memset(
                    bass_ap_3d(rt_all, 0, 66, NKT * HPC, 1), 0.0)
                nc.gpsimd.memset(
                    bass_ap_3d(rt_all, 65, 66, NKT * HPC, 1), 0.0)
                nc.gpsimd.dma_start(wo_sb[0:DH, :], wo[:])
                nc.gpsimd.dma_start(wo_sb[DH:2 * DH, :], wo[:])
                # x chunks 1-3 follow the weights on the gpsimd queue
                for c in range(1, NQ):
                    x_chunk_dma(c, 0, nc.gpsimd)
                    x_chunk_dma(c, 1, nc.gpsimd)

                # ---------- projection filler units ----------
                def transpose_unit(c, jj):
                    # transposes d-tiles 2jj, 2jj+1 of chunk c
                    pt = ppp.tile([128, 1024], f16, tag="pp", name="pt")
                    for j2 in range(2):
                        j = jj * 2 + j2
                        for st4 in range(4):
                            col = c * 4096 + st4 * 1024 + j * 128
                            nc.tensor.transpose(
                                pt[:, j2 * 512 + st4 * 128:
                                   j2 * 512 + (st4 + 1) * 128],
                                xs_all[:, col:col + 128],
                                idt[:])
                    dst = bass_ap_3d(xt_all, (jj * 2) * S + c * 512, S, 2, 512)
                    nc.vector.tensor_copy(dst, bass_ap_3d(pt, 0, 512, 2, 512))

                def qkproj_unit(c, which, m):
                    # one m-tile (2 heads) of the Q or K projection of chunk c
                    w_all = wq_all if which == 0 else wk_all
                    dest = qt_all if which == 0 else kt_all
                    ps = ppp.tile([128, 512], f32, tag="pp", name="ps")
                    for k in range(NDT):
                        nc.tensor.matmul(
                            ps[:],
                            w_all[:, k * GD + m * 128: k * GD + (m + 1) * 128],
                            xt_all[:, k * S + c * 512: k * S + (c + 1) * 512],
                            start=(k == 0), stop=(k == NDT - 1))
                    dcols = dest[:, m * S + c * 512: m * S + (c + 1) * 512]
                    if which == 0:
                        nc.vector.tensor_scalar_add(dcols, ps[:],
                                                    bq_t[:, m:m + 1])
                    else:
                        nc.vector.tensor_copy(dcols, ps[:])

                def vproj_unit(c, st4):
                    st = c * 4 + st4
                    ps = ppp.tile([128, 512], f32, tag="pp", name="ps")
                    for k in range(NDT):
                        nc.tensor.matmul(
                            ps[:],
                            xt_all[:, k * S + st * 128: k * S + (st + 1) * 128],
                            wv_all[:, k * GD:(k + 1) * GD],
                            start=(k == 0), stop=(k == NDT - 1))
                    dst = bass_ap_3d(vt_all, st * 528 + 1, 66, HPC, DH)
                    srcap = bass_ap_3d(ps, 0, DH, HPC, DH)
                    nc.vector.tensor_copy(dst, srcap)
                    rdst = bass_ap_3d(rt_all, st * 528 + 1, 66, HPC, DH)
                    nc.vector.tensor_tensor(rdst, srcap, dst, op=ALU.subtract)

                def proj_units(c):
                    units = []
                    for jj in range(4):
                        units.append(lambda c=c, jj=jj: transpose_unit(c, jj))
                    for which in range(2):
                        for m in range(4):
                            units.append(lambda c=c, w=which, m=m:
                                         qkproj_unit(c, w, m))
                    for st4 in range(4):
                        units.append(lambda c=c, s=st4: vproj_unit(c, s))
                    return units

                # ---------- attention ----------
                def make_drain(qc, hp, zt0, zt1, pending, npairs):
                    """Deferred: last AVs of (qc,hp), then fused norm."""
                    def av(pp, pet):
                        jz = max(2 * pp - 4 * qc, 0)
                        q0 = jz * 128
                        for half, zt in ((0, zt0), (1, zt1)):
                            eap = bass_ap_3d(pet, half * 512 + q0,
                                             1024, 2, 512 - q0)
                            voff = (2 * pp) * 528 + (2 * hp + half) * 66
                            nc.tensor.matmul(
                                zt[:, q0:512],
                                bass_ap_3d(vt_all, voff, 528, 2, 66),
                                eap, start=(pp == 0), stop=False,
                                perf_mode=DR)
                            nc.tensor.matmul(
                                zt[:, q0:512],
                                bass_ap_3d(rt_all, voff, 528, 2, 66),
                                eap, start=False,
                                stop=(pp == npairs - 1),
                                perf_mode=DR)

                    def drain():
                        while pending:
                            av(*pending.pop(0))
                        # fused normalization: 1/l straight off PSUM row 0,
                        # gpsimd broadcast, multiply into zsum rows 1-64
                        zcols = slice(qc * 512, (qc + 1) * 512)
                        for half, zt in ((0, zt0), (1, zt1)):
                            lrcp = lrp.tile([1, 512], f32, tag="lrcp")
                            nc.vector.reciprocal_approx_fast(
                                out=lrcp[:], in_=zt[0:1, :])
                            lbi = lbip.tile([DH + 1, 512], f32, tag="lbi")
                            nc.gpsimd.partition_broadcast(
                                lbi[:], lrcp[:], channels=DH + 1)
                            # DVE PSUM reads must sit at partition base 0:
                            # span rows 0-64 (row 0 computes l*(1/l) into
                            # the unused zsum row 0)
                            if 2 * hp + half == 0:
                                nc.vector.tensor_tensor(
                                    zsum[0:DH + 1, zcols], zt[0:DH + 1, :],
                                    lbi[0:DH + 1, :], op=ALU.mult)
                            else:
                                zn = znp.tile([DH + 1, 512], f32, tag="zn")
                                nc.vector.tensor_tensor(
                                    zn[0:DH + 1, :], zt[0:DH + 1, :],
                                    lbi[0:DH + 1, :], op=ALU.mult)
                                nc.vector.tensor_tensor(
                                    zsum[0:DH + 1, zcols],
                                    zsum[0:DH + 1, zcols],
                                    zn[0:DH + 1, :], op=ALU.add)
                    return drain

                def attention(qc, carry, filler, posts):
                    """carry: deferred drain from the previous (qc,hp);
                    filler: proj units to interleave; posts: deferred
                    tail-projection units of the previous chunk."""
                    ktiles = 4 * qc + 4
                    npairs = ktiles // 2
                    for hp in range(4):
                        zt0 = ztp.tile([66, 512], f32, tag="zt", name="zt0")
                        zt1 = ztp.tile([66, 512], f32, tag="zt", name="zt1")
                        pending = []

                        def av_flush(lag):
                            while len(pending) > lag:
                                pp, pet = pending.pop(0)
                                jz = max(2 * pp - 4 * qc, 0)
                                q0 = jz * 128
                                for half, zt in ((0, zt0), (1, zt1)):
                                    eap = bass_ap_3d(pet, half * 512 + q0,
                                                     1024, 2, 512 - q0)
                                    voff = ((2 * pp) * 528
                                            + (2 * hp + half) * 66)
                                    nc.tensor.matmul(
                                        zt[:, q0:512],
                                        bass_ap_3d(vt_all, voff, 528, 2, 66),
                                        eap, start=(pp == 0), stop=False,
                                        perf_mode=DR)
                                    nc.tensor.matmul(
                                        zt[:, q0:512],
                                        bass_ap_3d(rt_all, voff, 528, 2, 66),
                                        eap, start=False,
                                        stop=(pp == npairs - 1),
                                        perf_mode=DR)

                        et2 = None
                        for kt in range(ktiles):
                            sub = kt % 2
                            if sub == 0:
                                et2 = etp.tile([128, 2048], f8, tag="et",
                                               name="et")
                            base = sub * 1024
                            st2 = stp.tile([128, 1024], f32, tag="st2",
                                           name="st2")
                            j = kt - 4 * qc
                            q0 = max(j, 0) * 128
                            nc.tensor.matmul(
                                st2[:, q0:512],
                                kt_all[0:64, hp * S + kt * 128:
                                       hp * S + (kt + 1) * 128],
                                qt_all[0:64, hp * S + qc * 512 + q0:
                                       hp * S + (qc + 1) * 512],
                                start=True, stop=True, tile_position=(0, 0))
                            nc.tensor.matmul(
                                st2[:, 512 + q0:1024],
                                kt_all[64:128, hp * S + kt * 128:
                                       hp * S + (kt + 1) * 128],
                                qt_all[64:128, hp * S + qc * 512 + q0:
                                       hp * S + (qc + 1) * 512],
                                start=True, stop=True, tile_position=(64, 0))
                            if j > 0:
                                if sub == 1:
                                    nc.gpsimd.memset(
                                        bass_ap_3d(et2, base + (j - 1) * 128,
                                                   512, 2, 128), 0.0)
                                nc.scalar.activation(
                                    bass_ap_3d(et2, base + j * 128, 512, 2,
                                               512 - j * 128),
                                    bass_ap_3d(st2, j * 128, 512, 2,
                                               512 - j * 128),
                                    AF.Exp, scale=0.125)
                            else:
                                nc.scalar.activation(
                                    bass_ap_3d(et2, base, 512, 2, 512),
                                    st2[:], AF.Exp, scale=0.125)
                            if j >= 0:
                                for half in range(2):
                                    blk = et2[:, base + half * 512 + j * 128:
                                              base + half * 512 + (j + 1) * 128]
                                    nc.gpsimd.affine_select(
                                        out=blk, in_=blk, compare_op=ALU.is_ge,
                                        fill=0.0, base=0, pattern=[[1, 128]],
                                        channel_multiplier=-1)
                            if sub == 1:
                                pending.append((kt // 2, et2))
                            if kt == 2:
                                if carry[0] is not None:
                                    carry[0]()
                                    carry[0] = None
                                if posts:
                                    posts.pop(0)()
                            elif kt == 3 and filler:
                                filler.pop(0)()
                            elif kt == 5 and posts:
                                posts.pop(0)()
                            if sub == 1:
                                av_flush(3)
                        av_flush(2)
                        carry[0] = make_drain(qc, hp, zt0, zt1, pending,
                                              npairs)

                def tail_units(qc, last=False):
                    """Out-projection of chunk qc as 4 deferred units."""
                    zsr = zrp.tile([128, 512], f16, tag="zsr",
                                   name=f"zsr{qc}")
                    state = {"prepped": False}

                    def prep():
                        # both zsr halves via partition-shifting DMAs
                        # (zsum rows 1-64 -> zsr rows 0-63 / 64-127)
                        zc = zsum[1:DH + 1, qc * 512:(qc + 1) * 512]
                        nc.gpsimd.dma_start(zsr[0:DH, :], zc)
                        nc.gpsimd.dma_start(zsr[DH:2 * DH, :], zc)

                    def unit(qp):
                        if not state["prepped"]:
                            prep()
                            state["prepped"] = True
                        for sub2 in range(2):
                            for nn in range(2):
                                po = ppp.tile([128, 512], f32, tag="pp",
                                              name="po")
                                if sub2 == 0:
                                    nc.tensor.matmul(
                                        po[:],
                                        zsr[0:DH,
                                            (2 * qp) * 128:(2 * qp + 1) * 128],
                                        wo_sb[0:DH, nn * 512:(nn + 1) * 512],
                                        start=True, stop=True,
                                        tile_position=(0, 0))
                                else:
                                    nc.tensor.matmul(
                                        po[:],
                                        zsr[DH:128, (2 * qp + 1) * 128:
                                            (2 * qp + 2) * 128],
                                        wo_sb[DH:128, nn * 512:(nn + 1) * 512],
                                        start=True, stop=True,
                                        tile_position=(64, 0))
                                osb = osbp.tile([128, 512], f16, tag="osb")
                                if last:
                                    # scalar engine is exp-free by now
                                    nc.scalar.activation(osb[:], po[:],
                                                         AF.Copy)
                                else:
                                    nc.vector.tensor_copy(osb[:], po[:])
                                r0 = qc * 512 + (2 * qp) * 128 + sub2 * 128
                                nc.gpsimd.dma_start(
                                    out[r0:r0 + 128,
                                        nn * 512:(nn + 1) * 512],
                                    osb[:])
                    return [lambda qp=qp: unit(qp) for qp in range(2)]

                # ---------- main schedule ----------
                carry = [None]
                filler = []
                posts = []
                for u in proj_units(0):
                    u()
                for qc in range(NQ):
                    if qc + 1 < NQ:
                        filler.extend(proj_units(qc + 1))
                    attention(qc, carry, filler, posts)
                    # drain leftover proj filler between chunks
                    for u in filler:
                        u()
                    filler = []
                    if qc > 0:
                        # any tail units of chunk qc-1 not yet consumed
                        for u in posts:
                            u()
                        posts = []
                    posts.extend(tail_units(qc, last=(qc == NQ - 1)))
                # final chunk: drain + its tail immediately
                carry[0]()
                carry[0] = None
                for u in posts:
                    u()
    nc.compile()
    return nc


def kernel(**inputs):
    x = np.asarray(inputs["x"], dtype=np.float32)
    WQ = np.asarray(inputs["WQ"], dtype=np.float32)
    bQ = np.asarray(inputs["bQ"], dtype=np.float32)
    WK = np.asarray(inputs["WK"], dtype=np.float32)
    WV = np.asarray(inputs["WV"], dtype=np.float32)
    bV = np.asarray(inputs["bV"], dtype=np.float32)
    WO = np.asarray(inputs["WO"], dtype=np.float32)
    bO = np.asarray(inputs["bO"], dtype=np.float32)

    from concourse.bass_utils import run_bass_kernel_spmd

    if "nc" not in _prog:
        _prog["nc"] = _build()
    nc = _prog["nc"]

    in_maps = []
    for c in range(NCORES):
        b, g = c // 2, c % 2
        sl = slice(g * GD, (g + 1) * GD)
        in_maps.append({
            "x": np.ascontiguousarray(x[b]).astype(np.float16),
            "wq": np.ascontiguousarray(WQ[:, sl]).astype(np.float16),
            "wk": np.ascontiguousarray(WK[:, sl]).astype(np.float16),
            "wv": np.ascontiguousarray(WV[:, sl]).astype(np.float16),
            "bq": np.ascontiguousarray(bQ[sl]).reshape(1, GD).astype(np.float16),
            "wo": WO.astype(np.float16),
        })
    _prog["in_maps"] = in_maps
    globals()["_last_in_maps"] = in_maps
    res = run_bass_kernel_spmd(nc, in_maps, core_ids=list(range(NCORES)))
    _prog["res"] = res
    parts = [r["out"].astype(np.float32) for r in res.results]

    extra = bV.reshape(H, DH).sum(0) @ WO + np.float32(H) * bO
    out = np.empty((B, S, D), dtype=np.float32)
    for b in range(B):
        out[b] = parts[2 * b] + parts[2 * b + 1] + extra
    return out
